# revision 8
# baseline (speedup 1.0000x reference)
"""ChebConv Bass kernel v2: vertex-sharded, 8-batch-fused gathers + PE scatter.

Each core owns 12544 rows (98 tiles of 128, dealt round-robin over degree-
sorted tiles for load balance). Gather granule = one vertex row holding all
8 batches x 32 features in bf16 (512B). Edges are sorted by (tile, bucket,
row, col), padded to 128-edge chunks (dense, no slot padding). Per chunk a
precomputed [128e x 128r] bf16 scatter matrix (vals folded in) accumulates
y_tile = sum W_chunk^T @ z_chunk on the PE into PSUM fp32. Chebyshev
recurrence in fp32; bf16 copies of each step's local rows are AllGathered
into the next step's full gather table. Final einsum over K on the PE.
"""
import sys
import numpy as np

if '/opt/trn_rl_repo' not in sys.path:
    sys.path.insert(0, '/opt/trn_rl_repo')

import ml_dtypes

P = 128
V = 100000
FIN = 32
K = 4
FOUT = 64
B = 8
NCORE = 8
VPAD = 100352                 # 8 * 12544 = 4 * 25088 = 784 * 128
RPC = VPAD // NCORE           # 12544 rows per core
NTL = RPC // P                # 98 tiles per core
NBK = 4
BK = VPAD // NBK              # 25088 (int16-addressable granules)
F256 = B * FIN                # 256 columns (b, f)


def _preprocess(lap_rows, lap_cols, lap_vals):
    lap_rows = np.asarray(lap_rows).astype(np.int64)
    lap_cols = np.asarray(lap_cols).astype(np.int64)
    vals = np.asarray(lap_vals).astype(np.float32)
    E = len(lap_rows)

    deg = np.bincount(lap_rows, minlength=VPAD)
    order_by_deg = np.argsort(deg, kind="stable")   # rank -> old id
    rank = np.empty(VPAD, dtype=np.int64)
    rank[order_by_deg] = np.arange(VPAD)
    # rank r -> global tile j=r//128 -> core j%8, local tile j//8
    r = np.arange(VPAD)
    j = r // P
    pos_of_rank = (j % NCORE) * RPC + (j // NCORE) * P + (r % P)
    pos_of_old = pos_of_rank[rank]                  # old id -> table position

    Rp = pos_of_old[lap_rows]
    Cp = pos_of_old[lap_cols]

    # buckets = tile groups [0:25),[25:49),[49:74),[74:98) of the SOURCE row
    TGS = np.array([0, 25, 49, 74, 98])
    bucket_of_tile = np.searchsorted(TGS, np.arange(NTL), side="right") - 1
    rows_of_bucket = (TGS[1:] - TGS[:-1]) * P          # per-core rows
    c_src = Cp // RPC
    loc_src = Cp % RPC
    t_src = loc_src // P
    p_src = loc_src % P
    bkt = bucket_of_tile[t_src]
    idx_in_bucket = (c_src * rows_of_bucket[bkt]
                     + (t_src - TGS[bkt]) * P + p_src)

    # per (core, tile, bucket) counts -> uniform padded chunk counts
    core = Rp // RPC
    rl = Rp % RPC
    tile = rl // P
    bucket = bkt

    cnt = np.zeros((NCORE, NTL, NBK), dtype=np.int64)
    np.add.at(cnt, (core, tile, bucket), 1)
    NCH = (np.ceil(cnt.max(axis=0) / P).astype(np.int64))  # [NTL, NBK]
    TNCH = NCH.sum(axis=1)                                  # [NTL]
    total_ch = int(NCH.sum())

    # padded global chunk base per (tile, bucket)
    seg_base = np.zeros((NTL, NBK), dtype=np.int64)
    acc = 0
    for t in range(NTL):
        for b in range(NBK):
            seg_base[t, b] = acc
            acc += int(NCH[t, b])

    idx_cols = total_ch * 8                                 # int16 cols
    per_core = []
    for c in range(NCORE):
        sel = core == c
        Rl = rl[sel]
        Cg = idx_in_bucket[sel]
        Vv = vals[sel]
        tl = Rl // P
        bk = bucket[sel]
        o = np.lexsort((Cg, Rl, bk, tl))
        Rl, Cg, Vv, tl, bk = Rl[o], Cg[o], Vv[o], tl[o], bk[o]

        # rank within segment
        key = tl * NBK + bk
        is_start = np.r_[True, key[1:] != key[:-1]]
        starts = np.flatnonzero(is_start)
        run_len = np.diff(np.r_[starts, len(key)])
        rk = np.arange(len(key)) - np.repeat(starts, run_len)
        gp = seg_base[tl, bk] * P + rk                      # padded edge pos

        idx_flat = np.zeros(total_ch * P, dtype=np.int16)
        idx_flat[gp] = Cg.astype(np.int16)
        # compact W: per tile 16 idx/val columns (2 groups of <=8 chunks)
        val_c = np.zeros((P, NTL * 24), dtype=np.float32)
        rl_c = -np.ones((P, NTL * 24), dtype=np.int16)
        ch = gp // P          # global chunk
        pp = gp % P
        m = ch - seg_base[tl, 0]              # chunk local to tile
        g = m // 8
        col = tl * 24 + m
        val_c[pp, col] = Vv
        rl_c[pp, col] = ((m - g * 8) * P + (Rl % P)).astype(np.int16)
        w_core = (val_c.astype(ml_dtypes.bfloat16), rl_c)

        # per-segment wrap: flat[g] -> [g%16, segbase*8 + g//16]; one wrap
        iw = idx_flat.reshape(total_ch * 8, 16).T           # [16, total_ch*8]
        idx_core = np.ascontiguousarray(iw)
        per_core.append((idx_core, w_core))

    meta = dict(NCH=NCH, TNCH=TNCH, seg_base=seg_base, idx_cols=idx_cols,
                total_ch=total_ch, pos_of_old=pos_of_old,
                TGS=TGS, rows_of_bucket=rows_of_bucket)
    return meta, per_core


def _build_kernel(meta):
    import os
    import concourse.mybir as mybir
    import concourse.tile as tile
    from concourse import bacc
    from concourse.masks import make_identity

    NOCC = os.environ.get("K2_NOCC") == "1"
    NOEIN = os.environ.get("K2_NOEIN") == "1"
    NOGATH = os.environ.get("K2_NOGATH") == "1"

    f32 = mybir.dt.float32
    bf16 = mybir.dt.bfloat16
    i16 = mybir.dt.int16
    NCH = meta["NCH"]
    TNCH = meta["TNCH"]
    total_ch = meta["total_ch"]

    nc = bacc.Bacc(num_devices=8)

    x0locbf = nc.dram_tensor("x0locbf", [RPC, F256], bf16,
                             kind="ExternalInput")
    idx_all = nc.dram_tensor("idx_all", [16, total_ch * 8], i16,
                             kind="ExternalInput")
    val_all = nc.dram_tensor("val_all", [P, NTL * 24], bf16,
                             kind="ExternalInput")
    rl_all = nc.dram_tensor("rl_all", [P, NTL * 24], i16,
                            kind="ExternalInput")
    w4 = nc.dram_tensor("w4", [K * FIN, FOUT], f32, kind="ExternalInput")
    bias = nc.dram_tensor("bias", [P, FOUT], f32, kind="ExternalInput")
    out = nc.dram_tensor("out", [NTL, B, P, FOUT], bf16,
                         kind="ExternalOutput")

    TGS = meta["TGS"]
    ROB = meta["rows_of_bucket"]
    x0full = [nc.dram_tensor(f"x0fulli{j}", [NCORE * int(ROB[j]), F256], bf16)
              for j in range(NBK)]
    x0bounce = nc.dram_tensor("x0bounce", [RPC, F256], bf16)
    w_all = nc.dram_tensor("w_alli", [P, total_ch * P], bf16)
    x0loc32 = nc.dram_tensor("x0loc32", [RPC, F256], f32)
    xloc32 = [x0loc32] + [nc.dram_tensor(f"x{k}loc32", [RPC, F256], f32)
                          for k in (1, 2)]
    xbf = {k: nc.dram_tensor(f"x{k}bf", [RPC, F256], bf16) for k in (1, 2)}
    xfull = {k: [nc.dram_tensor(f"x{k}full{j}",
                                [NCORE * int(ROB[j]), F256], bf16)
                 for j in range(NBK)]
             for k in (1, 2)}

    groups = [list(range(8))]

    with tile.TileContext(nc) as tc:
        with (
            tc.tile_pool(name="const", bufs=1) as constp,
            tc.tile_pool(name="wp", bufs=4) as wp,
            tc.tile_pool(name="z", bufs=6) as zp,
            tc.tile_pool(name="io", bufs=4) as iop,
            tc.tile_pool(name="y", bufs=3) as yp,
            tc.tile_pool(name="xf", bufs=3) as xfp,
            tc.tile_pool(name="psy", bufs=3, space="PSUM") as psyp,
            tc.tile_pool(name="prh", bufs=2, space="PSUM") as prp,
            tc.tile_pool(name="pso", bufs=2, space="PSUM") as pop,
        ):
            ident = constp.tile([P, P], f32)
            make_identity(nc, ident[:])
            idx_t = constp.tile([P, total_ch * 8], i16)
            for r in range(8):
                nc.sync.dma_start(out=idx_t[16 * r:16 * (r + 1), :],
                                  in_=idx_all[:])
            w4k = []
            for k in range(K):
                wk = constp.tile([FIN, FOUT], f32, tag=f"w4k{k}")
                nc.sync.dma_start(out=wk[:],
                                  in_=w4[k * FIN:(k + 1) * FIN, :])
                w4k.append(wk)
            bias_t = constp.tile([P, FOUT], f32)
            nc.sync.dma_start(out=bias_t[:], in_=bias[:])

            # x0: AllGather bf16 shards into the full gather table
            nc.sync.dma_start(out=x0bounce[:], in_=x0locbf[:])
            if not NOCC:
                for j in range(NBK):
                    nc.gpsimd.collective_compute(
                        "AllGather", mybir.AluOpType.bypass,
                        replica_groups=groups,
                        ins=[x0bounce[int(TGS[j]) * P:int(TGS[j + 1]) * P, :]],
                        outs=[x0full[j][:]])
            # x0 local fp32 staging (recurrence + einsum)
            TB0 = 7
            for t0 in range(0, NTL, TB0):
                nb = min(TB0, NTL - t0)
                xbt = iop.tile([P, TB0, F256], bf16, tag="x0b")
                nc.sync.dma_start(
                    out=xbt[:, :nb, :],
                    in_=x0locbf[t0 * P:(t0 + nb) * P, :].rearrange(
                        "(t p) f -> p t f", p=P))
                xft = yp.tile([P, TB0, F256], f32, tag="x0f")
                nc.vector.tensor_copy(out=xft[:, :nb, :],
                                      in_=xbt[:, :nb, :])
                nc.sync.dma_start(
                    out=x0loc32[t0 * P:(t0 + nb) * P, :].rearrange(
                        "(t p) f -> p t f", p=P),
                    in_=xft[:, :nb, :])
            # W expansion: compact (val, colidx) -> dense chunks in HBM
            val_t = constp.tile([P, NTL * 24], bf16)
            nc.sync.dma_start(out=val_t[:], in_=val_all[:])
            rl_t = constp.tile([P, NTL * 24], i16)
            nc.sync.dma_start(out=rl_t[:], in_=rl_all[:])
            for t in range(NTL):
                tnch = int(TNCH[t])
                if tnch == 0:
                    continue
                wexp = wp.tile([P, 24 * P], bf16, tag="wexp")
                for g in range((tnch + 7) // 8):
                    gsz = min(8, tnch - g * 8)
                    nc.gpsimd.local_scatter(
                        out_ap=wexp[:, g * 8 * P:(g * 8 + gsz) * P],
                        data_ap=val_t[:, t * 24 + g * 8:t * 24 + g * 8 + 8],
                        idxs_ap=rl_t[:, t * 24 + g * 8:t * 24 + g * 8 + 8],
                        channels=P, num_elems=gsz * P, num_idxs=8)
                woff = int(meta["seg_base"][t, 0])
                nc.sync.dma_start(
                    out=w_all[:, woff * P:(woff + tnch) * P],
                    in_=wexp[:, :tnch * P])

            for k in (1, 2, 3):
                src = {1: x0full, 2: xfull[1], 3: xfull[2]}[k]
                if NOCC:
                    src = x0full
                for t in range(NTL):
                    tnch = int(TNCH[t])
                    xk32 = yp.tile([P, F256], f32, tag="x32")
                    if tnch > 0 and not NOGATH:
                        wt = wp.tile([P, tnch * P], bf16, tag="w")
                        woff = int(meta["seg_base"][t, 0])
                        nc.sync.dma_start(
                            out=wt[:],
                            in_=w_all[:, woff * P:(woff + tnch) * P])
                        psy = psyp.tile([P, F256], f32, tag="psy")
                        ci = 0
                        for b in range(NBK):
                            nch = int(NCH[t, b])
                            if nch == 0:
                                continue
                            ioff = int(meta["seg_base"][t, b])
                            z = zp.tile([P, nch, F256], bf16, tag="z")
                            n_idx = nch * P
                            nc.gpsimd.dma_gather(
                                out_ap=z[:],
                                in_ap=src[b][:],
                                idxs_ap=idx_t[:, ioff * 8:(ioff + nch) * 8],
                                num_idxs=n_idx, num_idxs_reg=n_idx,
                                elem_size=F256,
                                single_packet=(n_idx <= 512),
                            )
                            for m in range(nch):
                                nc.tensor.matmul(
                                    out=psy[:],
                                    lhsT=wt[:, (ci + m) * P:(ci + m + 1) * P],
                                    rhs=z[:, m, :],
                                    start=(ci + m == 0),
                                    stop=(ci + m == tnch - 1),
                                )
                            ci += nch
                        if k == 1:
                            nc.vector.tensor_copy(out=xk32[:], in_=psy[:])
                        else:
                            xo = iop.tile([P, F256], f32, tag="xo")
                            nc.sync.dma_start(
                                out=xo[:],
                                in_=xloc32[k - 2][t * P:(t + 1) * P, :])
                            nc.vector.tensor_scalar_mul(
                                out=xk32[:], in0=psy[:], scalar1=2.0)
                            nc.vector.tensor_tensor(
                                out=xk32[:], in0=xk32[:], in1=xo[:],
                                op=mybir.AluOpType.subtract)
                    else:
                        nc.vector.memset(xk32[:], 0.0)
                    if k < 3:
                        nc.sync.dma_start(
                            out=xloc32[k][t * P:(t + 1) * P, :], in_=xk32[:])
                        xkb = yp.tile([P, F256], bf16, tag="xbf")
                        nc.vector.tensor_copy(out=xkb[:], in_=xk32[:])
                        nc.sync.dma_start(
                            out=xbf[k][t * P:(t + 1) * P, :], in_=xkb[:])
                    if k == 3:
                        xts = []
                        for k2 in range(3):
                            xt = xfp.tile([P, F256], f32, tag=f"xt{k2}")
                            nc.sync.dma_start(
                                out=xt[:],
                                in_=xloc32[k2][t * P:(t + 1) * P, :])
                            xts.append(xt)
                        xts.append(xk32)
                        xTb = []
                        for b in range(B):
                            xtb = xfp.tile([FIN, K * P], f32, tag=f"xTb{b}")
                            xTb.append(xtb)
                        for k2 in range(K):
                            for h in range(2):
                                pxh = prp.tile([P, P], f32, tag="pxh")
                                nc.tensor.transpose(
                                    out=pxh[:],
                                    in_=xts[k2][:, h * P:(h + 1) * P],
                                    identity=ident[:])
                                for jj in range(4):
                                    b = h * 4 + jj
                                    nc.vector.tensor_copy(
                                        out=xTb[b][:, k2 * P:(k2 + 1) * P],
                                        in_=pxh[jj * FIN:(jj + 1) * FIN, :])
                        for b in range(B):
                            ops = pop.tile([P, FOUT], f32, tag="ops")
                            for k2 in range(K):
                                nc.tensor.matmul(
                                    out=ops[:],
                                    lhsT=xTb[b][:, k2 * P:(k2 + 1) * P],
                                    rhs=w4k[k2][:],
                                    start=(k2 == 0), stop=(k2 == K - 1))
                            ot = yp.tile([P, FOUT], bf16, tag="ot")
                            nc.vector.tensor_tensor(out=ot[:], in0=ops[:],
                                                    in1=bias_t[:],
                                                    op=mybir.AluOpType.add)
                            nc.sync.dma_start(out=out[t, b], in_=ot[:])
                    if k < 3 and not NOCC and t + 1 in (25, 49, 74, 98):
                        j = {25: 0, 49: 1, 74: 2, 98: 3}[t + 1]
                        s, e = int(TGS[j]), int(TGS[j + 1])
                        nc.gpsimd.collective_compute(
                            "AllGather",
                            mybir.AluOpType.bypass,
                            replica_groups=groups,
                            ins=[xbf[k][s * P:e * P, :]],
                            outs=[xfull[k][j][:]],
                        )


    return nc


# ---------------- PJRT runner (self-contained) ----------------

def _make_runner(nc, n_cores=8):
    import jax
    from jax.sharding import Mesh, PartitionSpec
    from jax.experimental.shard_map import shard_map
    import concourse.mybir as mybir
    from concourse.bass2jax import (
        _bass_exec_p, install_neuronx_cc_hook, partition_id_tensor)

    install_neuronx_cc_hook()
    if not nc.is_finalized():
        nc.finalize()
    partition_name = (nc.partition_id_tensor.name
                      if nc.partition_id_tensor else None)

    in_names, out_names, out_avals, zero_outs = [], [], [], []
    for alloc in nc.m.functions[0].allocations:
        if not isinstance(alloc, mybir.MemoryLocationSet):
            continue
        name = alloc.memorylocations[0].name
        if alloc.kind == "ExternalInput":
            if name != partition_name:
                in_names.append(name)
        elif alloc.kind == "ExternalOutput":
            out_names.append(name)
            shape = tuple(alloc.tensor_shape)
            dtype = mybir.dt.np(alloc.dtype)
            out_avals.append(jax.core.ShapedArray(shape, dtype))
            zero_outs.append(np.zeros(shape, dtype))
    n_params = len(in_names)
    all_in_names = in_names + out_names
    if partition_name is not None:
        all_in_names = all_in_names + [partition_name]

    def _body(*args):
        operands = list(args)
        if partition_name is not None:
            operands.append(partition_id_tensor())
        outs = _bass_exec_p.bind(
            *operands,
            out_avals=tuple(out_avals),
            in_names=tuple(all_in_names),
            out_names=tuple(out_names),
            lowering_input_output_aliases=(),
            sim_require_finite=True,
            sim_require_nnan=True,
            nc=nc,
        )
        return tuple(outs)

    devices = jax.devices()[:n_cores]
    mesh = Mesh(np.asarray(devices), ("core",))
    in_specs = (PartitionSpec("core"),) * (n_params + len(out_names))
    out_specs = (PartitionSpec("core"),) * len(out_names)
    sharded = jax.jit(
        shard_map(_body, mesh=mesh, in_specs=in_specs, out_specs=out_specs,
                  check_rep=False),
        keep_unused=True,
    )

    def run(in_maps):
        per_core = [[np.asarray(m[nm]) for nm in in_names] for m in in_maps]
        concat_in = [
            np.concatenate([per_core[c][i] for c in range(n_cores)], axis=0)
            for i in range(n_params)
        ]
        concat_zeros = [
            np.zeros((n_cores * z.shape[0], *z.shape[1:]), z.dtype)
            for z in zero_outs
        ]
        args = [jax.device_put(a) for a in concat_in + concat_zeros]
        outs = sharded(*args)
        jax.block_until_ready(outs)
        return [
            {nm: np.asarray(outs[i]).reshape(n_cores, *out_avals[i].shape)[c]
             for i, nm in enumerate(out_names)}
            for c in range(n_cores)
        ], (sharded, args)

    return run



_CACHE = {}
_LAST_RUN_STATE = None


def _get_built(lap_rows, lap_cols, lap_vals):
    if "k" not in _CACHE:
        meta, per_core = _preprocess(lap_rows, lap_cols, lap_vals)
        nc = _build_kernel(meta)
        run = _make_runner(nc, 8)
        _CACHE["k"] = (meta, per_core, run)
    return _CACHE["k"]


def kernel(inputs, lap_rows, lap_cols, lap_vals, weight, bias):
    global _LAST_RUN_STATE
    inputs = np.asarray(inputs)
    weight = np.asarray(weight)
    bias = np.asarray(bias)

    meta, per_core, run = _get_built(lap_rows, lap_cols, lap_vals)
    pos_of_old = meta["pos_of_old"]

    xf = np.zeros((VPAD, F256), dtype=np.float32)
    xf[pos_of_old[np.arange(V)]] = (
        inputs.transpose(1, 0, 2).reshape(V, F256))
    x0bf = xf.astype(ml_dtypes.bfloat16)

    w4 = np.ascontiguousarray(
        weight.transpose(1, 0, 2).reshape(K * FIN, FOUT)).astype(np.float32)
    bias_c = np.ascontiguousarray(
        np.broadcast_to(bias.reshape(1, FOUT), (P, FOUT))).astype(np.float32)

    in_maps = []
    for c in range(NCORE):
        idx_core, (val_c, rl_c) = per_core[c]
        in_maps.append(dict(
            x0locbf=np.ascontiguousarray(x0bf[c * RPC:(c + 1) * RPC]),
            idx_all=idx_core,
            val_all=val_c,
            rl_all=rl_c,
            w4=w4,
            bias=bias_c,
        ))

    res, _LAST_RUN_STATE = run(in_maps)

    # out tensors: [NTL, B, FOUT, P] per core; pos = c*RPC + t*128 + p
    full = np.empty((B, VPAD, FOUT), dtype=np.float32)
    for c in range(NCORE):
        o = np.asarray(res[c]["out"]).astype(np.float32)
        o = o.transpose(1, 0, 2, 3).reshape(B, RPC, FOUT)
        full[:, c * RPC:(c + 1) * RPC, :] = o
    out = np.empty((B, V, FOUT), dtype=np.float32)
    out[:] = full[:, pos_of_old[np.arange(V)], :]
    return out
